# revision 1
# baseline (speedup 1.0000x reference)
"""Causal self-attention (B=4, T=2048, C=768, H=12) on 8 Trainium2 cores.

Sharding: core c handles batch b=c//2 and heads [6*(c%2), 6*(c%2)+6).
Each core computes its 6 heads end-to-end (qkv proj -> attention -> partial
c_proj); the host sums the two partial c_proj outputs per batch and adds
b_proj (tensor-parallel all-reduce done host-side).

Matmuls run in float32r (fp32 storage, reduced-precision multiplies at 4x
the fp32 PE rate); accumulation stays fp32 in PSUM.

Attention per head with S computed transposed (keys on partitions):
  S.T[k,q] = K.T @ Q per 128-key block; expS = exp(S.T * 1/sqrt(D)) fused on
  ScalarE (no max subtraction -- scores are bounded, exp stays in fp32 range);
  causal mask via affine_select on the diagonal block only;
  O'[65,q] += V'[kblock].T @ expS in PSUM, where V' carries an all-ones
  column so O'[64] accumulates the softmax denominators for free;
  O = O'[0:64] * recip(O'[64]) with a stream_shuffle partition broadcast.

Emission interleaves q/k projection with attention heads so ScalarE exp work
overlaps projection PE work (PSUM split: st 4 banks, O' 2, projections 2).
"""

import sys

sys.path.insert(0, "/opt/trn_rl_repo")

from contextlib import ExitStack

import numpy as np

import concourse.bass as bass
import concourse.tile as tile
from concourse import bacc, mybir, bass_utils

B, T, C, H = 4, 2048, 768, 12
D = C // H  # 64
HPC = H // 2  # heads per core = 6
NCORES = 8
QKC = 2 * HPC * D  # 768 q+k outcols per core
VC = HPC * (D + 1)  # 390 v cols (64 v + 1 ones per head)
KB = T // 128  # 16 key blocks
TB = T // 128  # 16 token blocks
CB = C // 128  # 6 contraction chunks
HT = T // 2  # 1024, q-half width

f32 = mybir.dt.float32
f32r = mybir.dt.float32r
ts = bass.ts
SCALE = 1.0 / float(np.sqrt(D))


def _emit(tc, xT, wqk, bqk, wv, wpc, y, dbg=None):
    nc = tc.nc
    Exp = mybir.ActivationFunctionType.Exp

    with ExitStack() as top:
        qkTp = top.enter_context(tc.tile_pool(name="qkTp", bufs=1))
        vtp = top.enter_context(tc.tile_pool(name="vtp", bufs=1))
        ocp = top.enter_context(tc.tile_pool(name="ocp", bufs=1))
        wp = top.enter_context(tc.tile_pool(name="wp", bufs=1))
        esp = top.enter_context(tc.tile_pool(name="esp", bufs=4))
        nrm = top.enter_context(tc.tile_pool(name="nrm", bufs=1))
        ohp = top.enter_context(tc.tile_pool(name="ohp", bufs=2))

        qkt = [qkTp.tile([128, T], f32r, tag=f"qkt{i}", name=f"qkt{i}") for i in range(CB)]
        vt = [vtp.tile([128, VC], f32r, tag=f"vt{t}", name=f"vt{t}") for t in range(TB)]
        ocat = [ocp.tile([128, T], f32r, tag=f"oc{i}", name=f"oc{i}") for i in range(3)]

        wqkt = [wp.tile([128, QKC], f32r, tag=f"wqk{i}", name=f"wqk{i}") for i in range(CB)]
        bqt = [wp.tile([128, 1], f32, tag=f"bq{i}", name=f"bq{i}") for i in range(CB)]

        def emit_qk_proj(ob, qkps):
            # qkT[128*ob : 128*(ob+1), :] = W@x.T + bias, token chunks of 512
            for tch in range(4):
                ps = qkps.tile([128, 512], f32, tag="qkps", name="qkps")
                for kc in range(CB):
                    nc.tensor.matmul(
                        ps[:],
                        wqkt[kc][:, ts(ob, 128)],
                        xt[kc][:, ts(tch, 512)],
                        start=(kc == 0),
                        stop=(kc == CB - 1),
                    )
                nc.vector.tensor_scalar_add(
                    qkt[ob][:, ts(tch, 512)], ps[:], bqt[ob][:, 0:1]
                )

        def norm_half(h, op, half):
            # O = O'[0:64] / O'[64] for q-half `half`; write into ocat
            bp = 64 * (h % 2)
            rb = nrm.tile([96, HT], f32, tag="rb", name="rb")
            nc.vector.tensor_copy(rb[D : D + 1, :], op[D : D + 1, :])
            nc.vector.stream_shuffle(rb[32:64, :], rb[64:96, :], mask=[0] * 32)
            nc.gpsimd.tensor_copy(rb[0:32, :], rb[32:64, :])
            rb2 = nrm.tile([64, HT], f32, tag="rb2", name="rb2")
            nc.vector.reciprocal_approx_fast(out=rb2[:], in_=rb[0:D, :])
            oh = ohp.tile([64, HT], f32r, tag="oh", name="oh")
            nc.vector.tensor_mul(oh[:], op[0:D, :], rb2[:])
            nc.sync.dma_start(
                ocat[h // 2][bp : bp + 64, half * HT : half * HT + HT], oh[:]
            )

        def emit_head(h, stp, opp):
            qt = qkt[h // 2]
            kt = qkt[3 + h // 2]
            bp = 64 * (h % 2)
            for half in range(2):
                hlo = half * HT
                hhi = hlo + HT
                op = opp.tile([D + 1, HT], f32, tag="op", name="op")
                for kb in range(min(KB, (hhi + 127) // 128)):
                    qs = 128 * kb
                    lo = max(hlo, qs)
                    if lo >= hhi:
                        continue
                    st = stp.tile([128, HT], f32, tag="st", name="st")
                    es = esp.tile([128, HT], f32r, tag="es", name="es")
                    # QK: S.T[k, q] in pieces of <=512 within one PSUM bank
                    a = lo
                    while a < hhi:
                        b = min((a // 512 + 1) * 512, hhi)
                        nc.tensor.matmul(
                            st[:, a - hlo : b - hlo],
                            kt[bp : bp + 64, ts(kb, 128)],
                            qt[bp : bp + 64, a:b],
                            start=True,
                            stop=True,
                        )
                        a = b
                    nc.scalar.activation(
                        es[:, lo - hlo : HT], st[:, lo - hlo : HT], Exp, scale=SCALE
                    )
                    diag = hlo <= qs
                    if diag:
                        # causal mask on the diagonal block: keep q >= k
                        nc.gpsimd.affine_select(
                            out=es[:, qs - hlo : qs - hlo + 128],
                            in_=es[:, qs - hlo : qs - hlo + 128],
                            compare_op=mybir.AluOpType.is_ge,
                            fill=0.0,
                            base=0,
                            pattern=[[1, 128]],
                            channel_multiplier=-1,
                        )
                    # PV accumulate into O' (q-half slice); the diagonal
                    # 128-wide piece is split out so the rest doesn't wait
                    # on the mask
                    pieces = []
                    for qc in range(max(kb // 4, 2 * half), 2 * half + 2):
                        a = max(qs, 512 * qc)
                        b = 512 * (qc + 1)
                        if diag and a == qs and b > qs + 256:
                            # split so the non-diagonal part doesn't wait on
                            # the mask; start=True only on the first piece --
                            # it clears the whole bank's has_written bits
                            pieces.append((qs, qs + 256, kb == 0, False))
                            pieces.append((qs + 256, b, False, kb == 4 * qc + 3))
                        else:
                            pieces.append((a, b, kb == 0, kb == 4 * qc + 3))
                    for a, b, start_f, stop_f in pieces:
                        nc.tensor.matmul(
                            op[:, a - hlo : b - hlo],
                            vt[kb][:, 65 * h : 65 * h + 65],
                            es[:, a - hlo : b - hlo],
                            start=start_f,
                            stop=stop_f,
                        )
                norm_half(h, op, half)

        # ---------------- emission: projections interleaved with heads ----
        with ExitStack() as psA:
            stp = psA.enter_context(tc.tile_pool(name="stp", bufs=2, space="PSUM"))
            op1 = psA.enter_context(ExitStack())
            opp = op1.enter_context(tc.tile_pool(name="opp", bufs=1, space="PSUM"))

            with ExitStack() as xsc:
                xw = xsc.enter_context(tc.tile_pool(name="xw", bufs=1))
                xt = [
                    xw.tile([128, T], f32r, tag=f"xt{i}", name=f"xt{i}")
                    for i in range(CB)
                ]
                with ExitStack() as wvsc:
                    wvp = wvsc.enter_context(tc.tile_pool(name="wvp", bufs=1))
                    wvt = [
                        wvp.tile([128, VC], f32r, tag=f"wv{i}", name=f"wv{i}")
                        for i in range(CB)
                    ]
                    ones128 = wvp.tile([1, 128], f32r, tag="ones128", name="ones128")
                    nc.sync.dma_start(ones128[:], xT[C : C + 1, 0:128])
                    wvb = wvp.tile([1, VC], f32r, tag="wvb", name="wvb")
                    nc.sync.dma_start(wvb[:], wv[C : C + 1, :])
                    for i in range(CB):
                        nc.sync.dma_start(wvt[i][:], wv[ts(i, 128), :])
                        nc.sync.dma_start(bqt[i][:], bqk[ts(i, 128), :])
                    for tch in range(4):
                        for i in range(CB):
                            nc.sync.dma_start(
                                xt[i][:, ts(tch, 512)], xT[ts(i, 128), ts(tch, 512)]
                            )
                        if tch < 2:
                            for i in range(3 * tch, 3 * tch + 3):
                                nc.sync.dma_start(wqkt[i][:], wqk[ts(i, 128), :])

                    with tc.tile_pool(name="vps", bufs=2, space="PSUM") as vps:
                        for tb in range(TB):
                            ps = vps.tile([128, VC], f32, tag="vps", name="vps")
                            for kc in range(CB + 1):
                                if kc < CB:
                                    lhsT = xt[kc][:, ts(tb, 128)]
                                    rhs = wvt[kc][:]
                                else:
                                    lhsT = ones128[:, 0:128]
                                    rhs = wvb[:]
                                nc.tensor.matmul(
                                    ps[:], lhsT, rhs, start=(kc == 0), stop=(kc == CB)
                                )
                            nc.vector.tensor_copy(vt[tb][:], ps[:])

                with tc.tile_pool(name="qkps", bufs=2, space="PSUM") as qkps:
                    emit_qk_proj(0, qkps)
                    emit_qk_proj(3, qkps)
                    emit_head(0, stp, opp)
                    emit_qk_proj(1, qkps)
                    emit_qk_proj(4, qkps)
                    emit_head(1, stp, opp)
                    emit_qk_proj(2, qkps)
                    emit_qk_proj(5, qkps)

            op1.close()
            with tc.tile_pool(name="opp2", bufs=2, space="PSUM") as opp2:
                emit_head(2, stp, opp2)
                emit_head(3, stp, opp2)
                emit_head(4, stp, opp2)
                emit_head(5, stp, opp2)

        if dbg is not None:
            for i in range(CB):
                nc.sync.dma_start(dbg["qkT"][ts(i, 128), :], qkt[i][:].bitcast(f32))
            for t in range(TB):
                nc.sync.dma_start(dbg["v"][ts(t, 128), :], vt[t][:].bitcast(f32))
            for i in range(3):
                nc.sync.dma_start(dbg["oc"][ts(i, 128), :], ocat[i][:].bitcast(f32))

        # ---------------- output projection ----------------
        with ExitStack() as phC:
            wpp = phC.enter_context(tc.tile_pool(name="wpp", bufs=1))
            yop = phC.enter_context(tc.tile_pool(name="yop", bufs=3))
            yps = phC.enter_context(tc.tile_pool(name="yps", bufs=2, space="PSUM"))

            wpt = [wpp.tile([128, C], f32r, tag=f"wp{i}", name=f"wp{i}") for i in range(3)]
            for i in range(3):
                nc.sync.dma_start(wpt[i][:], wpc[ts(i, 128), :])

            for tb in range(TB):
                ps = yps.tile([128, C], f32, tag="yps", name="yps")
                for kc in range(3):
                    for a, w in ((0, 512), (512, 256)):
                        nc.tensor.matmul(
                            ps[:, a : a + w],
                            ocat[kc][:, ts(tb, 128)],
                            wpt[kc][:, a : a + w],
                            start=(kc == 0),
                            stop=(kc == 2),
                        )
                yt = yop.tile([128, C], f32, tag="yt", name="yt")
                if tb % 2 == 0:
                    nc.vector.tensor_copy(yt[:], ps[:])
                else:
                    nc.scalar.activation(
                        yt[:], ps[:], mybir.ActivationFunctionType.Copy
                    )
                nc.sync.dma_start(y[ts(tb, 128), :], yt[:])


_PROGRAM = None


def _build():
    global _PROGRAM
    if _PROGRAM is not None:
        return _PROGRAM
    nc = bacc.Bacc("TRN2", target_bir_lowering=False, debug=False, num_devices=NCORES)
    xT = nc.dram_tensor("xT", [C + 1, T], f32r, kind="ExternalInput").ap()
    wqk = nc.dram_tensor("wqk", [C, QKC], f32r, kind="ExternalInput").ap()
    bqk = nc.dram_tensor("bqk", [QKC, 1], f32, kind="ExternalInput").ap()
    wv = nc.dram_tensor("wv", [C + 1, VC], f32r, kind="ExternalInput").ap()
    wpc = nc.dram_tensor("wpc", [HPC * D, C], f32r, kind="ExternalInput").ap()
    y = nc.dram_tensor("y", [T, C], f32, kind="ExternalOutput").ap()
    with tile.TileContext(nc) as tc:
        _emit(tc, xT, wqk, bqk, wv, wpc, y)
    nc.compile()
    _PROGRAM = nc
    return nc


def _in_maps(x, w_qkv, b_qkv, w_proj):
    maps = []
    for c in range(NCORES):
        b = c // 2
        half = c % 2
        h0 = HPC * half  # first global head
        r0 = D * h0  # row offset within each of q/k/v sections
        span = HPC * D  # 384

        xTb = np.vstack([x[b].T, np.ones((1, T), np.float32)])  # [C+1, T]

        wq = w_qkv[r0 : r0 + span, :]
        wk = w_qkv[C + r0 : C + r0 + span, :]
        wqk = np.ascontiguousarray(np.vstack([wq, wk]).T)  # [C, 768]
        bqk = np.concatenate(
            [b_qkv[r0 : r0 + span], b_qkv[C + r0 : C + r0 + span]]
        ).reshape(QKC, 1)

        wv = np.zeros((C + 1, VC), dtype=np.float32)
        for hl in range(HPC):
            g = 2 * C + r0 + D * hl
            wv[0:C, 65 * hl : 65 * hl + D] = w_qkv[g : g + D, :].T
            wv[C, 65 * hl : 65 * hl + D] = b_qkv[g : g + D]
            wv[C, 65 * hl + D] = 1.0

        wpc = np.ascontiguousarray(w_proj[:, r0 : r0 + span].T)  # [384, C]

        maps.append(
            {
                "xT": xTb.astype(np.float32),
                "wqk": wqk.astype(np.float32),
                "bqk": bqk.astype(np.float32),
                "wv": wv,
                "wpc": wpc.astype(np.float32),
            }
        )
    return maps


def kernel(x, w_qkv, b_qkv, w_proj, b_proj, _trace=False):
    x = np.asarray(x, dtype=np.float32)
    w_qkv = np.asarray(w_qkv, dtype=np.float32)
    b_qkv = np.asarray(b_qkv, dtype=np.float32)
    w_proj = np.asarray(w_proj, dtype=np.float32)
    b_proj = np.asarray(b_proj, dtype=np.float32)

    nc = _build()
    maps = _in_maps(x, w_qkv, b_qkv, w_proj)
    res = bass_utils.run_bass_kernel_spmd(
        nc, maps, core_ids=list(range(NCORES)), trace=_trace
    )
    out = np.empty((B, T, C), dtype=np.float32)
    for b in range(B):
        out[b] = res.results[2 * b]["y"] + res.results[2 * b + 1]["y"] + b_proj
    if _trace:
        kernel._last_exec_time_ns = res.exec_time_ns
        kernel._last_results = res
    return out

